# revision 11
# baseline (speedup 1.0000x reference)
# Self-contained Trainium2 Bass kernel for:
#   scores = Q @ K.T            [N, M]
#   attn   = softmax(scores, axis=0)   (over queries, per key column)
#   out    = attn @ V           [N, D]
# with N = M = 8192, D = 128, float32 I/O.
#
# Sharding: K/V rows (the M axis) are split across the 8 NeuronCores.
# The softmax axis (N) stays fully local to each core, so no collectives
# are needed: each core produces a partial out^T = sum over its M-shard,
# and the host sums the 8 partials.
#
# Device algorithm (per core, M_SH = 1024):
#   scoresT = K_sh @ Q^T        [M_SH, N]   (PE, fp16 inputs, f32 PSUM)
#   expT    = exp(scoresT)      bf16, via ScalarE directly from PSUM,
#                               with fused accum_out row-sums -> denom[m]
#   V'      = V / denom[:,None] bf16 (fold softmax normalizer into V)
#   outT    = V'^T @ expT       [D, N]      (PE, bf16, accumulated in PSUM)
#
# No max-subtraction is needed: scores ~ N(0, 128), |s| < ~70, and
# exp(70) ~ 2.5e30 fits fp32/bf16 range comfortably.
#
# Layouts: the contraction dim of phase 1 is D=128, which must sit on the
# SBUF partition axis for the PE; the host passes Q^T and K_sh^T so every
# DMA is a contiguous load and the device never transposes anything.

from contextlib import ExitStack

import numpy as np

import concourse.bass as bass
import concourse.mybir as mybir
import concourse.tile as tile
from concourse import bacc
from concourse.bass_utils import run_bass_kernel_spmd

N, M, D = 8192, 8192, 128
N_CORES = 8
M_SH = M // N_CORES  # 1024

F32 = mybir.dt.float32
F16 = mybir.dt.float16
BF16 = mybir.dt.bfloat16


def build_attention_nc(n=N, m_sh=M_SH, d=D, mm_chunk=512, exp_chunk=2048):
    """Build the per-core Bass program.

    mm_chunk: free-dim (n) size of each phase-1/phase-2 matmul (<=512, one
              f32 PSUM bank per matmul).
    exp_chunk: free-dim size of each ScalarE exp op; one PSUM tile of
               exp_chunk/mm_chunk banks is filled by that many matmuls and
               consumed by a single activation instruction.
    """
    assert d == 128
    assert m_sh % 128 == 0 and n % exp_chunk == 0 and exp_chunk % mm_chunk == 0
    MT = m_sh // 128           # m-tiles of 128 partitions
    ECH = n // exp_chunk       # exp chunks per m-tile
    MM_PER_E = exp_chunk // mm_chunk
    NCH = n // mm_chunk        # phase-2 output chunks

    nc = bacc.Bacc()
    qt = nc.dram_tensor("qt", [d, n], F16, kind="ExternalInput")
    kt = nc.dram_tensor("kt", [d, m_sh], F16, kind="ExternalInput")
    v = nc.dram_tensor("v", [m_sh, d], F32, kind="ExternalInput")
    ot = nc.dram_tensor("ot", [d, n], F32, kind="ExternalOutput")

    with tile.TileContext(nc) as tc, ExitStack() as ctx:
        singles = ctx.enter_context(tc.tile_pool(name="singles", bufs=1))
        # One PSUM pool; phase-1 exp tiles and phase-2 accumulators share the
        # same tag so 2 slots x 4 banks = all 8 banks, no overlap conflict.
        psum = ctx.enter_context(tc.tile_pool(name="psum", bufs=2, space="PSUM"))
        outp = ctx.enter_context(tc.tile_pool(name="outp", bufs=3))

        # kt first (small, needed by the very first matmul), then qt in
        # chunks so matmul 0 isn't gated on the full 2 MB load, v last.
        kt_s = singles.tile([d, m_sh], F16)
        nc.sync.dma_start(out=kt_s, in_=kt[:, :])
        qt_s = singles.tile([d, n], F16)
        n_ld = max(exp_chunk, n // 8)
        for i in range(n // n_ld):
            nc.sync.dma_start(
                out=qt_s[:, i * n_ld : (i + 1) * n_ld],
                in_=qt[:, i * n_ld : (i + 1) * n_ld],
            )
        v_s = singles.tile([128, MT, d], F32)
        nc.sync.dma_start(out=v_s, in_=v.rearrange("(t p) d -> p t d", p=128))
        # First-touch v_s on DVE: the TS (tensor_scalar) instruction format
        # has a single HW sync-wait slot, so the real consumer below must not
        # be the one that waits on this DMA.
        v_touch = singles.tile([128, 1], F32)
        nc.vector.tensor_copy(v_touch, v_s[:, 0, 0:1])

        expT = [
            singles.tile([128, n], BF16, tag=f"expT{mt}", name=f"expT{mt}")
            for mt in range(MT)
        ]
        dch = [
            singles.tile([128, ECH], F32, tag=f"dch{mt}", name=f"dch{mt}")
            for mt in range(MT)
        ]
        denom = singles.tile([128, MT], F32)
        recip = singles.tile([128, MT], F32)
        vb = singles.tile([128, MT, d], BF16)

        # ---- Phase 1: scoresT = K_sh @ Q^T, exp, row-sums ----
        for mt in range(MT):
            k_col = kt_s[:, mt * 128 : (mt + 1) * 128]
            for e in range(ECH):
                ps = psum.tile([128, exp_chunk], F32, tag="ps", name="ps")
                for j in range(MM_PER_E):
                    c0 = e * exp_chunk + j * mm_chunk
                    nc.tensor.matmul(
                        ps[:, j * mm_chunk : (j + 1) * mm_chunk],
                        lhsT=k_col,
                        rhs=qt_s[:, c0 : c0 + mm_chunk],
                        start=True,
                        stop=True,
                    )
                nc.scalar.activation(
                    out=expT[mt][:, e * exp_chunk : (e + 1) * exp_chunk],
                    in_=ps,
                    func=mybir.ActivationFunctionType.Exp,
                    accum_out=dch[mt][:, e : e + 1],
                )
            nc.vector.reduce_sum(
                denom[:, mt : mt + 1], dch[mt][:, :], axis=mybir.AxisListType.X
            )
            nc.vector.reciprocal(recip[:, mt : mt + 1], denom[:, mt : mt + 1])
            nc.vector.tensor_scalar_mul(
                vb[:, mt, :], v_s[:, mt, :], recip[:, mt : mt + 1]
            )

        # ---- Phase 2: outT = V'^T @ expT, accumulated over m-tiles ----
        for c in range(NCH):
            ps2 = psum.tile([128, mm_chunk], F32, tag="ps", name="ps2")
            for mt in range(MT):
                nc.tensor.matmul(
                    ps2,
                    lhsT=vb[:, mt, :],
                    rhs=expT[mt][:, c * mm_chunk : (c + 1) * mm_chunk],
                    start=(mt == 0),
                    stop=(mt == MT - 1),
                )
            o_s = outp.tile([128, mm_chunk], F32)
            nc.vector.tensor_copy(o_s, ps2)
            nc.sync.dma_start(out=ot[:, c * mm_chunk : (c + 1) * mm_chunk], in_=o_s)

    nc.compile()
    return nc


_CACHE = {}


def _get_nc():
    if "nc" not in _CACHE:
        _CACHE["nc"] = build_attention_nc()
    return _CACHE["nc"]


def make_in_maps(Q, K, V):
    Q = np.asarray(Q, dtype=np.float32)
    K = np.asarray(K, dtype=np.float32)
    V = np.asarray(V, dtype=np.float32)
    qt = np.ascontiguousarray(Q.T.astype(np.float16))
    in_maps = []
    for i in range(N_CORES):
        sl = slice(i * M_SH, (i + 1) * M_SH)
        in_maps.append(
            {
                "qt": qt,
                "kt": np.ascontiguousarray(K[sl].T.astype(np.float16)),
                "v": np.ascontiguousarray(V[sl]),
            }
        )
    return in_maps


def combine_results(per_core_ot):
    acc = np.zeros((D, N), dtype=np.float64)
    for o in per_core_ot:
        acc += o.astype(np.float64)
    return np.ascontiguousarray(acc.T).astype(np.float32)


def kernel(Q, K, V):
    in_maps = make_in_maps(Q, K, V)
    res = run_bass_kernel_spmd(_get_nc(), in_maps, core_ids=list(range(N_CORES)))
    return combine_results([r["ot"] for r in res.results])
